# revision 1
# baseline (speedup 1.0000x reference)
"""Trainium2 Bass kernel for ExemplarImageMatching.

Math (per batch b):
  ei  = relu(bn1(W_img @ x))            x = image[b] as [C, HW]
  A   = s2*(Wa @ ei) + (s2*b_dr + t2)   (bn2 scale folded; Wa = W_dr[:, :C])
  ee  = relu(W_ex @ ex_b^T + b_ex)
  D   = s2*(Wb @ ee)                    (Wb = W_dr[:, C:])
  sim[n, f] = sum_c relu(A[c, f] + D[c, n])^2
  out = softmax(sim / TEMP, axis=f)

Sharding: data-parallel over B across the 8 cores (B == 8), one image per
core; the N loop runs on-core.  BN/bias folding and the hi/lo tf32 weight
splits happen on host.

Precision: the softmax logits are sim/0.1 with a per-row spread of ~500, so
the softmax is nearly saturated and sim must be fp32-accurate (a single tf32
rounding anywhere costs ~2e-2 final error).  Two tricks keep fp32 accuracy
at speed:
 - GEMMs run as 3-term Karatsuba-style f32r (tf32) matmuls:
   W@x ~= Wh@xh + Wh@xl + Wl@xh with W = Wh+Wl, x = xh+xl split hi/lo at
   11 mantissa bits.  f32r matmuls run at 1 cycle/row vs 4 for fp32, so 3
   terms cost 3/4 of an fp32 matmul; measured end-to-end error 7.5e-5.
   x is split on host; ei is split on device (ACT copy with f32r output
   rounds; DVE subtract with f32r output forms the residual).
 - The channel sum runs on the PE as full-fp32 one-hot-column matmuls
   (4 cycles/row); the two 128-channel blocks are pre-added ("fold") so the
   PE only sums half the tiles.

Engine layout of the big pass ((chunk, n) "groups", engines-per-lane kept
closed so cross-engine waits stay rare):
  hybrid lane: ACT relu-add (bias trick) -> DVE square -> DVE fold -> PE
  gpsimd lane: relu-add/square/fold all on GPSIMD -> PE
GEMM work for each chunk is emitted just before the chunk's groups so PE
GEMM matmuls overlap the previous chunk's elementwise work.

The per-n channel sum lands on PSUM partition row n via a "one-hot column"
stationary matrix (PE output partition offsets must be quadrant-aligned, so
we write all 16 rows and let rows != n accumulate zeros).

Softmax: per-chunk partial maxima accumulate off the critical path (read
straight from PSUM).  Chunks 0..NCH-2 exponentiate in place against the
provisional max while the last chunk is still computing; a scalar factor
gamma = exp(10*(M3 - M)) corrects their denominators, and per-chunk
normalize+DMA pipelines the stores.
"""

from contextlib import ExitStack

import numpy as np

import concourse.bass as bass
import concourse.bacc as bacc
import concourse.tile as tile
from concourse import mybir
from concourse.bass_utils import run_bass_kernel_spmd

B, N, C, H, W = 8, 16, 256, 64, 64
HW = H * W
P = 128
CB = C // P            # channel blocks (2)
FT = 512               # matmul free-dim tile (one PSUM bank of fp32)
FC = 1024              # f-chunk for the big elementwise pass
NCH = HW // FC         # 4
NFI = FC // FT         # 2
FPC = FC // FT         # f-tiles per chunk
EPS = 1e-5
INV_TEMP = 10.0

F32 = mybir.dt.float32
F32R = mybir.dt.float32r
AF = mybir.ActivationFunctionType
OP = mybir.AluOpType
AX = mybir.AxisListType.X

# Lane per (chunk, n) group: "hyb" (ACT relu-add + DVE square/fold) or
# "gps" (everything on GPSIMD).  First group of each chunk is "hyb".
_CHUNK_LANES = ["hyb", "gps", "hyb", "gps", "hyb", "hyb", "gps", "hyb",
                "gps", "hyb", "hyb", "gps", "hyb", "gps", "hyb", "gps"]
LANE = _CHUNK_LANES * NCH


def _build_nc():
    nc = bacc.Bacc()

    xh_d = nc.dram_tensor("xh", [C, HW], F32R, kind="ExternalInput")
    xl_d = nc.dram_tensor("xl", [C, HW], F32R, kind="ExternalInput")
    wimgTh_d = nc.dram_tensor("wimgTh", [C, C], F32R, kind="ExternalInput")
    wimgTl_d = nc.dram_tensor("wimgTl", [C, C], F32R, kind="ExternalInput")
    waTh_d = nc.dram_tensor("waTh", [C, C], F32R, kind="ExternalInput")
    waTl_d = nc.dram_tensor("waTl", [C, C], F32R, kind="ExternalInput")
    waTf_d = nc.dram_tensor("waTf", [C, C], F32, kind="ExternalInput")
    wexT_d = nc.dram_tensor("wexT", [C, C], F32, kind="ExternalInput")
    wbT_d = nc.dram_tensor("wbT", [C, C], F32, kind="ExternalInput")
    exT_d = nc.dram_tensor("exT", [C, N], F32, kind="ExternalInput")
    bei_d = nc.dram_tensor("bei", [P, CB], F32, kind="ExternalInput")
    bA_d = nc.dram_tensor("bA", [P, CB], F32, kind="ExternalInput")
    bex_d = nc.dram_tensor("bex", [P, CB], F32, kind="ExternalInput")
    out_d = nc.dram_tensor("out", [N, HW], F32, kind="ExternalOutput")

    with ExitStack() as ctx:
        tc = ctx.enter_context(tile.TileContext(nc))
        singles = ctx.enter_context(tc.tile_pool(name="singles", bufs=1))
        xpool = ctx.enter_context(tc.tile_pool(name="xpool", bufs=2))
        eipool = ctx.enter_context(tc.tile_pool(name="eipool", bufs=1))
        espool = ctx.enter_context(tc.tile_pool(name="espool", bufs=1))
        rh_pool = ctx.enter_context(tc.tile_pool(name="rh", bufs=2))
        sqh_pool = ctx.enter_context(tc.tile_pool(name="sqh", bufs=3))
        fh_pool = ctx.enter_context(tc.tile_pool(name="fh", bufs=2))
        rg_pool = ctx.enter_context(tc.tile_pool(name="rg", bufs=2))
        sqg_pool = ctx.enter_context(tc.tile_pool(name="sqg", bufs=2))
        fg_pool = ctx.enter_context(tc.tile_pool(name="fg", bufs=2))
        gps_pool = ctx.enter_context(tc.tile_pool(name="gps_ps", bufs=1, space="PSUM"))
        sim_pool = ctx.enter_context(tc.tile_pool(name="sim_ps", bufs=2, space="PSUM"))

        # ---- constants / weights -------------------------------------------------
        def load(dram_ap, shape, tag, dt=F32):
            t = singles.tile(shape, dt, tag=tag, name=tag)
            nc.sync.dma_start(t[:], dram_ap)
            return t

        rr = lambda d: d[:, :].rearrange("(cb p) o -> p cb o", p=P)
        exT = load(exT_d[:, :].rearrange("(cb p) n -> p cb n", p=P), [P, CB, N], "exT")
        wexT = load(rr(wexT_d), [P, CB, C], "wexT")
        wimgTh = load(rr(wimgTh_d), [P, CB, C], "wimgTh", F32R)
        bex = load(bex_d[:, :], [P, CB], "bex")
        bei = load(bei_d[:, :], [P, CB], "bei")
        wbT = load(rr(wbT_d), [P, CB, C], "wbT")
        wimgTl = load(rr(wimgTl_d), [P, CB, C], "wimgTl", F32R)
        waTh = load(rr(waTh_d), [P, CB, C], "waTh", F32R)
        waTl = load(rr(waTl_d), [P, CB, C], "waTl", F32R)
        waTf = load(rr(waTf_d), [P, CB, C], "waTf")
        bA = load(bA_d[:, :], [P, CB], "bA")

        # Z[:, N-1] = 1, rest 0.  Z[:, N-1-n : 2N-1-n] is a [P, N] matrix whose
        # column n is all-ones (DVE-produced).
        zsel = singles.tile([P, 2 * N - 1], F32)
        nc.vector.memset(zsel[:], 0.0)
        nc.vector.memset(zsel[:, N - 1:N], 1.0)

        # ---- exemplar branch: ee = relu(WexT.T @ exT + bex); D = WbT.T @ ee ------
        ee = singles.tile([P, CB, N], F32)
        for ob in range(CB):
            ps = gps_pool.tile([P, FT], F32, tag="g00", name=f"ee_ps{ob}")
            for cb in range(CB):
                nc.tensor.matmul(
                    ps[:, :N],
                    wexT[:, cb, ob * P:(ob + 1) * P],
                    exT[:, cb, :],
                    start=(cb == 0), stop=(cb == CB - 1),
                )
            nc.scalar.activation(ee[:, ob, :], ps[:, :N], AF.Relu, bias=bex[:, ob:ob + 1])
        Dt = singles.tile([P, CB, N], F32)
        for ob in range(CB):
            ps = gps_pool.tile([P, FT], F32, tag="g00", name=f"d_ps{ob}")
            for eb in range(CB):
                nc.tensor.matmul(
                    ps[:, :N],
                    wbT[:, eb, ob * P:(ob + 1) * P],
                    ee[:, eb, :],
                    start=(eb == 0), stop=(eb == CB - 1),
                )
            nc.scalar.copy(Dt[:, ob, :], ps[:, :N])

        # ---- per chunk: GEMMs then big pass --------------------------------------
        A_sb = singles.tile([P, CB, HW], F32)
        sim_sb = singles.tile([N, HW], F32)
        pmax = singles.tile([N, NCH], F32)
        xh_r = xh_d[:, :].rearrange("(cb p) hw -> p cb hw", p=P)
        xl_r = xl_d[:, :].rearrange("(cb p) hw -> p cb hw", p=P)

        for fc in range(NCH):
            f0 = fc * FC
            # GEMM1: ei = relu(Wimg' @ x + bei), 3-term f32r
            xh_t = xpool.tile([P, CB, FC], F32R, tag="xh", name=f"xh{fc}")
            for cb in range(CB):
                nc.sync.dma_start(xh_t[:, cb, :], xh_r[:, cb, f0:f0 + FC])
            xl_t = xpool.tile([P, CB, FC], F32R, tag="xl", name=f"xl{fc}")
            for cb in range(CB):
                nc.sync.dma_start(xl_t[:, cb, :], xl_r[:, cb, f0:f0 + FC])
            ps1 = [gps_pool.tile([P, FT], F32, tag=f"g{ob}{t2}", name=f"ps1_{fc}_{ob}{t2}")
                   for ob in range(CB) for t2 in range(FPC)]
            terms1 = [(wimgTh, xh_t), (wimgTh, xl_t), (wimgTl, xh_t)]
            nt = len(terms1)
            for ti, (wt, xt) in enumerate(terms1):
                for cb in range(CB):
                    for ob in range(CB):
                        for t2 in range(FPC):
                            nc.tensor.matmul(
                                ps1[ob * FPC + t2][:],
                                wt[:, cb, ob * P:(ob + 1) * P],
                                xt[:, cb, t2 * FT:(t2 + 1) * FT],
                                start=(ti == 0 and cb == 0),
                                stop=(ti == nt - 1 and cb == CB - 1),
                                skip_group_check=True,
                            )
            ei_t = eipool.tile([P, CB, FC], F32, tag="ei", name=f"ei{fc}")
            for ob in range(CB):
                for t2 in range(FPC):
                    nc.scalar.activation(
                        ei_t[:, ob, t2 * FT:(t2 + 1) * FT], ps1[ob * FPC + t2][:],
                        AF.Relu, bias=bei[:, ob:ob + 1])
            ps2 = [gps_pool.tile([P, FT], F32, tag=f"g{ob}{t2}", name=f"ps2_{fc}_{ob}{t2}")
                   for ob in range(CB) for t2 in range(FPC)]
            if fc == 0:
                # chunk 0: plain fp32 GEMM2 (exact) -- the pipeline is empty,
                # so skipping the ei hi/lo split chain shortens startup.
                for cb in range(CB):
                    for ob in range(CB):
                        for t2 in range(FPC):
                            nc.tensor.matmul(
                                ps2[ob * FPC + t2][:],
                                waTf[:, cb, ob * P:(ob + 1) * P],
                                ei_t[:, cb, t2 * FT:(t2 + 1) * FT],
                                start=(cb == 0), stop=(cb == CB - 1),
                                skip_group_check=True,
                            )
            else:
                # device hi/lo split of ei (ACT rounds to tf32, DVE residual)
                eih_t = espool.tile([P, CB, FC], F32R, tag="eih", name=f"eih{fc}")
                nc.scalar.copy(eih_t[:], ei_t[:])
                eil_t = espool.tile([P, CB, FC], F32R, tag="eil", name=f"eil{fc}")
                nc.vector.tensor_tensor(eil_t[:], ei_t[:], eih_t[:], op=OP.subtract)
                # GEMM2: A = Wa' @ ei + bA, 3-term f32r
                terms2 = [(waTh, eih_t), (waTh, eil_t), (waTl, eih_t)]
                for ti, (wt, et) in enumerate(terms2):
                    for cb in range(CB):
                        for ob in range(CB):
                            for t2 in range(FPC):
                                nc.tensor.matmul(
                                    ps2[ob * FPC + t2][:],
                                    wt[:, cb, ob * P:(ob + 1) * P],
                                    et[:, cb, t2 * FT:(t2 + 1) * FT],
                                    start=(ti == 0 and cb == 0),
                                    stop=(ti == nt - 1 and cb == CB - 1),
                                    skip_group_check=True,
                                )
            for ob in range(CB):
                for t2 in range(FPC):
                    nc.scalar.activation(
                        A_sb[:, ob, f0 + t2 * FT:f0 + (t2 + 1) * FT],
                        ps2[ob * FPC + t2][:], AF.Identity, bias=bA[:, ob:ob + 1])

            # big-pass chunk: r = relu(A + D_n); fsum = r0^2 + r1^2;
            # sim = zsel.T @ fsum
            sim_ps = sim_pool.tile([N, FC], F32, tag="sim", name=f"sim_ps{fc}")
            for n in range(N):
                lane = LANE[fc * N + n]
                if lane == "hyb":
                    sqs = []
                    for cb in range(CB):
                        r_t = rh_pool.tile([P, FC], F32, tag="rh", name=f"rh{fc}_{n}_{cb}")
                        nc.scalar.activation(
                            r_t[:], A_sb[:, cb, f0:f0 + FC], AF.Relu,
                            bias=Dt[:, cb, n:n + 1],
                        )
                        sq_t = sqh_pool.tile([P, FC], F32, tag="sqh", name=f"sqh{fc}_{n}_{cb}")
                        nc.vector.tensor_mul(sq_t[:], r_t[:], r_t[:])
                        sqs.append(sq_t)
                    fsum = fh_pool.tile([P, FC], F32, tag="fh", name=f"fh{fc}_{n}")
                    nc.vector.tensor_add(fsum[:], sqs[0][:], sqs[1][:])
                else:
                    sqs = []
                    for cb in range(CB):
                        r_t = rg_pool.tile([P, FC], F32, tag="rg", name=f"rg{fc}_{n}_{cb}")
                        nc.gpsimd.tensor_scalar(
                            r_t[:], A_sb[:, cb, f0:f0 + FC],
                            Dt[:, cb, n:n + 1], 0.0, op0=OP.add, op1=OP.max,
                        )
                        sq_t = sqg_pool.tile([P, FC], F32, tag="sqg", name=f"sqg{fc}_{n}_{cb}")
                        nc.gpsimd.tensor_mul(sq_t[:], r_t[:], r_t[:])
                        sqs.append(sq_t)
                    fsum = fg_pool.tile([P, FC], F32, tag="fg", name=f"fg{fc}_{n}")
                    nc.gpsimd.tensor_add(fsum[:], sqs[0][:], sqs[1][:])
                for fi in range(NFI):
                    nc.tensor.matmul(
                        sim_ps[:, fi * FT:(fi + 1) * FT],
                        zsel[:, N - 1 - n:2 * N - 1 - n],
                        fsum[:, fi * FT:(fi + 1) * FT],
                        start=(n == 0), stop=(n == N - 1),
                        skip_group_check=True,
                    )
            if fc < NCH - 1:
                nc.scalar.copy(sim_sb[:, f0:f0 + FC], sim_ps[:])
            else:
                last_sim_ps = sim_ps
            # chunk partial max, read from PSUM in parallel with the copy
            nc.vector.reduce_max(pmax[:, fc:fc + 1], sim_ps[:], axis=AX)

        # ---- softmax over hw (per n): logits = sim * 10 --------------------------
        # Early exp: chunks 0..NCH-2 exponentiate against the provisional max
        # M3 = max(pmax[0..NCH-2]) while chunk NCH-1 is still computing; the
        # final denominators are corrected by gamma = exp(10*(M3 - M)) <= 1.
        # (x <= M3 in those chunks, so exp(10*(x - M3)) never overflows.)
        m3 = singles.tile([N, 1], F32)
        nc.vector.reduce_max(m3[:], pmax[:, 0:NCH - 1], axis=AX)
        nm3 = singles.tile([N, 1], F32)
        nc.vector.tensor_scalar_mul(nm3[:], m3[:], -INV_TEMP)
        dens = singles.tile([N, NCH], F32)
        for fc in range(NCH - 1):
            nc.scalar.activation(
                sim_sb[:, fc * FC:(fc + 1) * FC], sim_sb[:, fc * FC:(fc + 1) * FC],
                AF.Exp, bias=nm3[:], scale=INV_TEMP, accum_out=dens[:, fc:fc + 1],
            )
        mx = singles.tile([N, 1], F32)
        nc.vector.reduce_max(mx[:], pmax[:], axis=AX)
        nmx = singles.tile([N, 1], F32)
        nc.vector.tensor_scalar_mul(nmx[:], mx[:], -INV_TEMP)
        lastc = (NCH - 1) * FC
        nc.scalar.activation(
            sim_sb[:, lastc:], last_sim_ps[:],
            AF.Exp, bias=nmx[:], scale=INV_TEMP, accum_out=dens[:, NCH - 1:NCH],
        )
        # gamma = exp(10*(m3 - M)): ACT Exp with scale 10 on (m3 - M)
        dm = singles.tile([N, 1], F32)
        nc.vector.tensor_tensor(dm[:], m3[:], mx[:], op=OP.subtract)
        gam = singles.tile([N, 1], F32)
        nc.scalar.activation(gam[:], dm[:], AF.Exp, scale=INV_TEMP)
        den012 = singles.tile([N, 1], F32)
        nc.vector.reduce_sum(den012[:], dens[:, 0:NCH - 1], axis=AX)
        den = singles.tile([N, 1], F32)
        # den = gamma*den012 + den_last
        nc.vector.scalar_tensor_tensor(
            den[:], in0=den012[:], scalar=gam[:], in1=dens[:, NCH - 1:NCH],
            op0=OP.mult, op1=OP.add,
        )
        rden = singles.tile([N, 1], F32)
        nc.vector.reciprocal(rden[:], den[:])
        grden = singles.tile([N, 1], F32)
        nc.vector.tensor_mul(grden[:], gam[:], rden[:])
        # normalize: chunks 0..NCH-2 scale by gamma/den, last chunk by 1/den;
        # per-chunk norm+DMA so the stores pipeline across queues
        nc.scalar.activation(sim_sb[:, lastc:], sim_sb[:, lastc:], AF.Copy, scale=rden[:])
        nc.sync.dma_start(out_d[:, lastc:], sim_sb[:, lastc:])
        for fc in range(NCH - 1):
            f0 = fc * FC
            nc.vector.tensor_scalar_mul(sim_sb[:, f0:f0 + FC],
                                        sim_sb[:, f0:f0 + FC], grden[:])
            nc.sync.dma_start(out_d[:, f0:f0 + FC], sim_sb[:, f0:f0 + FC])

    nc.compile()
    return nc


_NC_CACHE = {}


def _get_nc():
    if "nc" not in _NC_CACHE:
        _NC_CACHE["nc"] = _build_nc()
    return _NC_CACHE["nc"]


def _tf32(x):
    u = np.ascontiguousarray(x, dtype=np.float32).view(np.uint32)
    return ((u + np.uint32(0x1000)) & np.uint32(0xFFFFE000)).view(np.float32)


def _make_in_maps(inputs):
    f32 = np.float32
    img = np.ascontiguousarray(inputs["image_features"], dtype=f32)     # [B,C,H,W]
    ex = np.ascontiguousarray(inputs["exemplar_features"], dtype=f32)   # [B,N,C]

    s1 = (inputs["bn1_gamma"] / np.sqrt(inputs["bn1_var"] + EPS)).astype(f32)
    t1 = (inputs["bn1_beta"] - inputs["bn1_mean"] * s1).astype(f32)
    s2 = (inputs["bn2_gamma"] / np.sqrt(inputs["bn2_var"] + EPS)).astype(f32)
    t2 = (inputs["bn2_beta"] - inputs["bn2_mean"] * s2).astype(f32)

    W_img = np.asarray(inputs["W_img"], f32)
    W_dr = np.asarray(inputs["W_dr"], f32)
    W_ex = np.asarray(inputs["W_ex"], f32)

    wimg_f = s1[:, None] * W_img                       # [o, c]
    bei_full = (s1 * np.asarray(inputs["b_img"], f32) + t1).astype(f32)
    wa_f = s2[:, None] * W_dr[:, :C]
    bA_full = (s2 * np.asarray(inputs["b_dr"], f32) + t2).astype(f32)
    wb_f = s2[:, None] * W_dr[:, C:]
    bex_full = np.asarray(inputs["b_ex"], f32)

    def t(w):  # [o, c] -> [c, o], contiguous
        return np.ascontiguousarray(w.T.astype(f32))

    def pack_bias(v):  # [C] -> [P, CB], v[cb*P + p] at [p, cb]
        return np.ascontiguousarray(v.reshape(CB, P).T.astype(f32))

    def hl(w):  # hi/lo tf32 split
        h = _tf32(w)
        l = _tf32((w - h).astype(f32))
        return h, l

    wimgT = t(wimg_f)
    waT = t(wa_f)
    wimgTh, wimgTl = hl(wimgT)
    waTh, waTl = hl(waT)

    shared = {
        "wimgTh": wimgTh, "wimgTl": wimgTl,
        "waTh": waTh, "waTl": waTl, "waTf": waT,
        "wexT": t(W_ex),
        "wbT": t(wb_f),
        "bei": pack_bias(bei_full),
        "bA": pack_bias(bA_full),
        "bex": pack_bias(bex_full),
    }
    in_maps = []
    for b in range(B):
        m = dict(shared)
        x = np.ascontiguousarray(img[b].reshape(C, HW))
        xh = _tf32(x)
        xl = _tf32((x - xh).astype(f32))
        m["xh"] = xh
        m["xl"] = xl
        m["exT"] = np.ascontiguousarray(ex[b].T.astype(f32))
        in_maps.append(m)
    return in_maps


def _run(inputs, **kw):
    nc = _get_nc()
    in_maps = _make_in_maps(inputs)
    res = run_bass_kernel_spmd(nc, in_maps, core_ids=list(range(B)), **kw)
    out = np.stack([res.results[i]["out"] for i in range(B)])
    return out.reshape(B, N, H, W).astype(np.float32), res


def kernel(**inputs):
    out, _ = _run(inputs)
    return out



# revision 3
# speedup vs baseline: 1.4188x; 1.4188x over previous
"""Trainium2 Bass kernel for ExemplarImageMatching.

Math (per batch b):
  ei  = relu(bn1(W_img @ x))            x = image[b] as [C, HW]
  A   = s2*(Wa @ ei)                    (bn2 scale folded; Wa = W_dr[:, :C])
  ee  = relu(W_ex @ ex_b^T + b_ex)
  DA  = s2*(Wb @ ee) + (s2*b_dr + t2)   (per-n channel bias, [C, N])
  sim[n, f] = sum_c relu(A[c, f] + DA[c, n])^2
  out = softmax(sim / TEMP, axis=f)

Sharding: data-parallel over B across the 8 cores (B == 8), one image per
core; the N loop runs on-core.  BN/bias folding happens on host.

Everything is plain fp32 (no tf32 splits): exact numerics, ~5e-5 end-to-end.

The channel reduction sum_c rsq[c, f] runs on the PE with the SQUARED DATA AS
THE STATIONARY operand ([128c x 128f] tiles) and a ones-column as the moving
operand, producing a [128f, 1] column per (n, f-block).  Matmul time scales
with the moving pass (output free size), so these reductions are nearly free,
and they are exact fp32 (no one-hot fp32 moving pass, which costs 4 cyc/row
on the whole [*, 512] output).  The sums land transposed ([f, n]); small PE
transposes ([128,16] -> [16,128] against an identity) restore row-major sim
for the softmax.

Per (chunk, n) group the elementwise work is 2 passes over [C, FC]:
  RA: r = relu(A + DA_n)   -- tensor_scalar (add, max); DVE runs this in its
                              2x (dual-read-port) mode at ~0.5 elem/cycle
  SQ: rsq = r * r          -- tensor_tensor on Pool, or AF.Square on ACT
Groups are assigned engine lanes by a static table to balance DVE/Pool/ACT.

Emission order pipelines chunks: GEMM1(c+1) -> early groups(c) -> ei-relu(c+1)
-> GEMM2(c+1) -> mid groups(c) -> A-copy(c+1) -> late groups(c) -> transposes.
Engines execute their streams in emission order, so ACT's group squares are
emitted before its next-chunk epilogue copies.

Softmax: per-chunk partial maxima accumulate off the critical path.  Chunks
0..NCH-2 exponentiate against the provisional max M3 while the last chunk is
still computing; a scalar factor gamma = exp(10*(M3 - M)) corrects their
denominators, and per-chunk normalize+DMA pipelines the stores.
"""

from contextlib import ExitStack

import numpy as np

import concourse.bass as bass
import concourse.bacc as bacc
import concourse.tile as tile
from concourse import mybir
from concourse.bass_utils import run_bass_kernel_spmd

B, N, C, H, W = 8, 16, 256, 64, 64
HW = H * W
P = 128
CB = C // P            # channel blocks (2)
FT = 512               # matmul free-dim tile (one PSUM bank of fp32)
FC = 1024              # f-chunk for the big elementwise pass
NCH = HW // FC         # 4
NFB = FC // P          # f-blocks of 128 per chunk (8)
EPS = 1e-5
INV_TEMP = 10.0

F32 = mybir.dt.float32
AF = mybir.ActivationFunctionType
OP = mybir.AluOpType
AX = mybir.AxisListType.X

# Lane per (chunk-local group n): (ra_engine, sq_engine)
#   ra: "v" = DVE tensor_scalar (2x mode), "p" = Pool tensor_scalar
#   sq: "A" = ACT Square one 2D instr, "p" = Pool per-cb, "v" = DVE per-cb
_LANES = [
    ("v", "A"), ("v", "p"), ("v", "A"), ("v", "p"),
    ("v", "A"), ("v", "p"), ("v", "A"), ("v", "p"),
    ("v", "A"), ("v", "p"), ("v", "A"), ("v", "p"),
    ("p", "v"), ("p", "v"), ("p", "p"), ("p", "p"),
]


def _build_nc():
    nc = bacc.Bacc()

    x_d = nc.dram_tensor("x", [C, HW], F32, kind="ExternalInput")
    wimgT_d = nc.dram_tensor("wimgT", [C, C], F32, kind="ExternalInput")
    waT_d = nc.dram_tensor("waT", [C, C], F32, kind="ExternalInput")
    wexT_d = nc.dram_tensor("wexT", [C, C], F32, kind="ExternalInput")
    wbT_d = nc.dram_tensor("wbT", [C, C], F32, kind="ExternalInput")
    exT_d = nc.dram_tensor("exT", [C, N], F32, kind="ExternalInput")
    bei_d = nc.dram_tensor("bei", [P, CB], F32, kind="ExternalInput")
    bA_d = nc.dram_tensor("bA", [P, CB], F32, kind="ExternalInput")
    bex_d = nc.dram_tensor("bex", [P, CB], F32, kind="ExternalInput")
    ident_d = nc.dram_tensor("ident", [P, P], F32, kind="ExternalInput")
    out_d = nc.dram_tensor("out", [N, HW], F32, kind="ExternalOutput")

    with ExitStack() as ctx:
        tc = ctx.enter_context(tile.TileContext(nc))
        singles = ctx.enter_context(tc.tile_pool(name="singles", bufs=1))
        xpool = ctx.enter_context(tc.tile_pool(name="xpool", bufs=2))
        eipool = ctx.enter_context(tc.tile_pool(name="eipool", bufs=1))
        rpool = ctx.enter_context(tc.tile_pool(name="rp", bufs=4))
        sqpool = ctx.enter_context(tc.tile_pool(name="sqp", bufs=4))
        stpool = ctx.enter_context(tc.tile_pool(name="stp", bufs=2))
        gps_pool = ctx.enter_context(tc.tile_pool(name="gps_ps", bufs=1, space="PSUM"))
        st_ps_pool = ctx.enter_context(tc.tile_pool(name="st_ps", bufs=2, space="PSUM"))
        sim_pool = ctx.enter_context(tc.tile_pool(name="sim_ps", bufs=1, space="PSUM"))

        # ---- constants / weights -------------------------------------------------
        def load(dram_ap, shape, tag, dt=F32):
            t = singles.tile(shape, dt, tag=tag, name=tag)
            nc.sync.dma_start(t[:], dram_ap)
            return t

        rr = lambda d: d[:, :].rearrange("(cb p) o -> p cb o", p=P)
        exT = load(exT_d[:, :].rearrange("(cb p) n -> p cb n", p=P), [P, CB, N], "exT")
        wexT = load(rr(wexT_d), [P, CB, C], "wexT")
        wimgT = load(rr(wimgT_d), [P, CB, C], "wimgT")
        bex = load(bex_d[:, :], [P, CB], "bex")
        bei = load(bei_d[:, :], [P, CB], "bei")
        wbT = load(rr(wbT_d), [P, CB, C], "wbT")
        waT = load(rr(waT_d), [P, CB, C], "waT")
        bA = load(bA_d[:, :], [P, CB], "bA")
        ident = load(ident_d[:, :], [P, P], "ident")

        ones = singles.tile([P, 1], F32)
        nc.vector.memset(ones[:], 1.0)

        # ---- exemplar branch: ee = relu(WexT.T @ exT + bex); DA = WbT.T @ ee + bA
        ee = singles.tile([P, CB, N], F32)
        for ob in range(CB):
            ps = gps_pool.tile([P, FC], F32, tag="gA", name=f"ee_ps{ob}")
            for cb in range(CB):
                nc.tensor.matmul(
                    ps[:, :N],
                    wexT[:, cb, ob * P:(ob + 1) * P],
                    exT[:, cb, :],
                    start=(cb == 0), stop=(cb == CB - 1),
                )
            nc.scalar.activation(ee[:, ob, :], ps[:, :N], AF.Relu, bias=bex[:, ob:ob + 1])
        DA = singles.tile([P, CB, N], F32)
        for ob in range(CB):
            ps = gps_pool.tile([P, FC], F32, tag="gA", name=f"d_ps{ob}")
            for eb in range(CB):
                nc.tensor.matmul(
                    ps[:, :N],
                    wbT[:, eb, ob * P:(ob + 1) * P],
                    ee[:, eb, :],
                    start=(eb == 0), stop=(eb == CB - 1),
                )
            nc.scalar.activation(DA[:, ob, :], ps[:, :N], AF.Identity, bias=bA[:, ob:ob + 1])

        # ---- state ----------------------------------------------------------------
        A_sb = singles.tile([P, CB, HW], F32)
        sim_sb = singles.tile([N, HW], F32)
        pmax = singles.tile([N, NCH], F32)
        x_r = x_d[:, :].rearrange("(cb p) hw -> p cb hw", p=P)

        x_tiles = {}

        def emit_x_dma(c):
            t = xpool.tile([P, CB, FC], F32, tag="x", name=f"x{c}")
            for cb in range(CB):
                nc.sync.dma_start(t[:, cb, :], x_r[:, cb, c * FC:(c + 1) * FC])
            x_tiles[c] = t

        ps_gemm = {}

        def emit_g1(c):
            """GEMM1: ei_psum = Wimg' @ x, fp32."""
            pA = gps_pool.tile([P, FC], F32, tag="gA", name=f"ps1a_{c}")
            pB = gps_pool.tile([P, FC], F32, tag="gB", name=f"ps1b_{c}")
            ps_gemm[c] = (pA, pB)
            xt = x_tiles.pop(c)
            for cb in range(CB):
                for ob, pt in enumerate((pA, pB)):
                    for t2 in range(FC // FT):
                        nc.tensor.matmul(
                            pt[:, t2 * FT:(t2 + 1) * FT],
                            wimgT[:, cb, ob * P:(ob + 1) * P],
                            xt[:, cb, t2 * FT:(t2 + 1) * FT],
                            start=(cb == 0), stop=(cb == CB - 1),
                            skip_group_check=True,
                        )

        ei_tiles = {}

        def emit_eirelu(c):
            ei_t = eipool.tile([P, CB, FC], F32, tag="ei", name=f"ei{c}")
            pA, pB = ps_gemm.pop(c)
            for ob, pt in enumerate((pA, pB)):
                nc.scalar.activation(ei_t[:, ob, :], pt[:], AF.Relu, bias=bei[:, ob:ob + 1])
            ei_tiles[c] = ei_t

        def emit_g2(c):
            pA = gps_pool.tile([P, FC], F32, tag="gA", name=f"ps2a_{c}")
            pB = gps_pool.tile([P, FC], F32, tag="gB", name=f"ps2b_{c}")
            ps_gemm[c] = (pA, pB)
            ei_t = ei_tiles.pop(c)
            for cb in range(CB):
                for ob, pt in enumerate((pA, pB)):
                    for t2 in range(FC // FT):
                        nc.tensor.matmul(
                            pt[:, t2 * FT:(t2 + 1) * FT],
                            waT[:, cb, ob * P:(ob + 1) * P],
                            ei_t[:, cb, t2 * FT:(t2 + 1) * FT],
                            start=(cb == 0), stop=(cb == CB - 1),
                            skip_group_check=True,
                        )

        def emit_acopy(c):
            pA, pB = ps_gemm.pop(c)
            f0 = c * FC
            for ob, pt in enumerate((pA, pB)):
                nc.scalar.copy(A_sb[:, ob, f0:f0 + FC], pt[:])

        simT_ps = {}

        def emit_group(c, n):
            f0 = c * FC
            ra_eng, sq_eng = _LANES[n]
            ra = nc.vector if ra_eng == "v" else nc.gpsimd
            r_t = rpool.tile([P, CB, FC], F32, tag="r", name=f"r{c}_{n}")
            for cb in range(CB):
                ra.tensor_scalar(
                    r_t[:, cb, :], A_sb[:, cb, f0:f0 + FC],
                    DA[:, cb, n:n + 1], 0.0, op0=OP.add, op1=OP.max,
                )
            sq_t = sqpool.tile([P, CB, FC], F32, tag="sq", name=f"sq{c}_{n}")
            if sq_eng == "A":
                nc.scalar.activation(sq_t[:, :, :], r_t[:, :, :], AF.Square)
            else:
                eng = nc.vector if sq_eng == "v" else nc.gpsimd
                for cb in range(CB):
                    eng.tensor_tensor(sq_t[:, cb, :], r_t[:, cb, :], r_t[:, cb, :], op=OP.mult)
            # channel sums: stationary = squared data, moving = ones column.
            # (each cell's start/stop accumulation pair must be contiguous in
            # the PE stream, so cb is the inner loop)
            st = simT_ps[c]
            for fb in range(NFB):
                for cb in range(CB):
                    nc.tensor.matmul(
                        st[:, fb, n:n + 1],
                        sq_t[:, cb, fb * P:(fb + 1) * P],
                        ones[:, :],
                        start=(cb == 0), stop=(cb == CB - 1),
                        skip_group_check=True,
                    )

        sim_ps_tiles = {}

        def emit_chunk_epilogue(c):
            # simT [128f, NFB, N] -> SBUF -> 8 PE transposes -> sim_ps [N, FC]
            st = simT_ps.pop(c)
            st_sb = stpool.tile([P, NFB, N], F32, tag="st", name=f"st_sb{c}")
            nc.scalar.copy(st_sb[:, :, :], st[:, :, :])
            sim_ps_t = sim_pool.tile([N, FC], F32, tag="sim", name=f"sim_ps{c}")
            sim_ps_tiles[c] = sim_ps_t
            for fb in range(NFB):
                nc.tensor.transpose(
                    sim_ps_t[:, fb * P:(fb + 1) * P], st_sb[:, fb, :], ident[:, :],
                )
            # chunk partial max, read from PSUM in parallel with the copy
            nc.vector.reduce_max(pmax[:, c:c + 1], sim_ps_t[:], axis=AX)
            if c < NCH - 1:
                nc.scalar.copy(sim_sb[:, c * FC:(c + 1) * FC], sim_ps_t[:])
                sim_ps_tiles.pop(c)

        # ---- pipeline -------------------------------------------------------------
        emit_x_dma(0)
        emit_x_dma(1)
        emit_g1(0)
        emit_eirelu(0)
        emit_g2(0)
        emit_acopy(0)

        for c in range(NCH):
            simT_ps[c] = st_ps_pool.tile([P, NFB, N], F32, tag="st", name=f"simT{c}")
            nxt = c + 1
            if nxt < NCH:
                if nxt + 1 < NCH:
                    emit_x_dma(nxt + 1)
                emit_g1(nxt)
            for n in range(6):
                emit_group(c, n)
            if nxt < NCH:
                emit_eirelu(nxt)
                emit_g2(nxt)
            for n in range(6, 12):
                emit_group(c, n)
            if nxt < NCH:
                emit_acopy(nxt)
            for n in range(12, N):
                emit_group(c, n)
            emit_chunk_epilogue(c)

        # ---- softmax over hw (per n): logits = sim * 10 --------------------------
        # Early exp: chunks 0..NCH-2 exponentiate against the provisional max
        # M3 = max(pmax[0..NCH-2]) while chunk NCH-1 is still computing; the
        # final denominators are corrected by gamma = exp(10*(M3 - M)) <= 1.
        # (x <= M3 in those chunks, so exp(10*(x - M3)) never overflows.)
        m3 = singles.tile([N, 1], F32)
        nc.vector.reduce_max(m3[:], pmax[:, 0:NCH - 1], axis=AX)
        nm3 = singles.tile([N, 1], F32)
        nc.vector.tensor_scalar_mul(nm3[:], m3[:], -INV_TEMP)
        dens = singles.tile([N, NCH], F32)
        for fc in range(NCH - 1):
            nc.scalar.activation(
                sim_sb[:, fc * FC:(fc + 1) * FC], sim_sb[:, fc * FC:(fc + 1) * FC],
                AF.Exp, bias=nm3[:], scale=INV_TEMP, accum_out=dens[:, fc:fc + 1],
            )
        mx = singles.tile([N, 1], F32)
        nc.vector.reduce_max(mx[:], pmax[:], axis=AX)
        nmx = singles.tile([N, 1], F32)
        nc.vector.tensor_scalar_mul(nmx[:], mx[:], -INV_TEMP)
        lastc = (NCH - 1) * FC
        last_sim_ps = sim_ps_tiles.pop(NCH - 1)
        nc.scalar.activation(
            sim_sb[:, lastc:], last_sim_ps[:],
            AF.Exp, bias=nmx[:], scale=INV_TEMP, accum_out=dens[:, NCH - 1:NCH],
        )
        # gamma = exp(10*(m3 - M)): ACT Exp with scale 10 on (m3 - M)
        dm = singles.tile([N, 1], F32)
        nc.vector.tensor_tensor(dm[:], m3[:], mx[:], op=OP.subtract)
        gam = singles.tile([N, 1], F32)
        nc.scalar.activation(gam[:], dm[:], AF.Exp, scale=INV_TEMP)
        den012 = singles.tile([N, 1], F32)
        nc.vector.reduce_sum(den012[:], dens[:, 0:NCH - 1], axis=AX)
        den = singles.tile([N, 1], F32)
        # den = gamma*den012 + den_last
        nc.vector.scalar_tensor_tensor(
            den[:], in0=den012[:], scalar=gam[:], in1=dens[:, NCH - 1:NCH],
            op0=OP.mult, op1=OP.add,
        )
        rden = singles.tile([N, 1], F32)
        nc.vector.reciprocal(rden[:], den[:])
        grden = singles.tile([N, 1], F32)
        nc.vector.tensor_mul(grden[:], gam[:], rden[:])
        # normalize: chunks 0..NCH-2 scale by gamma/den, last chunk by 1/den;
        # per-chunk norm+DMA so the stores pipeline across queues
        nc.scalar.activation(sim_sb[:, lastc:], sim_sb[:, lastc:], AF.Copy, scale=rden[:])
        nc.sync.dma_start(out_d[:, lastc:], sim_sb[:, lastc:])
        for fc in range(NCH - 1):
            f0 = fc * FC
            nc.vector.tensor_scalar_mul(sim_sb[:, f0:f0 + FC],
                                        sim_sb[:, f0:f0 + FC], grden[:])
            nc.sync.dma_start(out_d[:, f0:f0 + FC], sim_sb[:, f0:f0 + FC])

    nc.compile()
    return nc


_NC_CACHE = {}


def _get_nc():
    if "nc" not in _NC_CACHE:
        _NC_CACHE["nc"] = _build_nc()
    return _NC_CACHE["nc"]


def _make_in_maps(inputs):
    f32 = np.float32
    img = np.ascontiguousarray(inputs["image_features"], dtype=f32)     # [B,C,H,W]
    ex = np.ascontiguousarray(inputs["exemplar_features"], dtype=f32)   # [B,N,C]

    s1 = (inputs["bn1_gamma"] / np.sqrt(inputs["bn1_var"] + EPS)).astype(f32)
    t1 = (inputs["bn1_beta"] - inputs["bn1_mean"] * s1).astype(f32)
    s2 = (inputs["bn2_gamma"] / np.sqrt(inputs["bn2_var"] + EPS)).astype(f32)
    t2 = (inputs["bn2_beta"] - inputs["bn2_mean"] * s2).astype(f32)

    W_img = np.asarray(inputs["W_img"], f32)
    W_dr = np.asarray(inputs["W_dr"], f32)
    W_ex = np.asarray(inputs["W_ex"], f32)

    wimg_f = s1[:, None] * W_img                       # [o, c]
    bei_full = (s1 * np.asarray(inputs["b_img"], f32) + t1).astype(f32)
    wa_f = s2[:, None] * W_dr[:, :C]
    bA_full = (s2 * np.asarray(inputs["b_dr"], f32) + t2).astype(f32)
    wb_f = s2[:, None] * W_dr[:, C:]
    bex_full = np.asarray(inputs["b_ex"], f32)

    def t(w):  # [o, c] -> [c, o], contiguous
        return np.ascontiguousarray(w.T.astype(f32))

    def pack_bias(v):  # [C] -> [P, CB], v[cb*P + p] at [p, cb]
        return np.ascontiguousarray(v.reshape(CB, P).T.astype(f32))

    shared = {
        "wimgT": t(wimg_f),
        "waT": t(wa_f),
        "wexT": t(W_ex),
        "wbT": t(wb_f),
        "bei": pack_bias(bei_full),
        "bA": pack_bias(bA_full),
        "bex": pack_bias(bex_full),
        "ident": np.eye(P, dtype=f32),
    }
    in_maps = []
    for b in range(B):
        m = dict(shared)
        m["x"] = np.ascontiguousarray(img[b].reshape(C, HW))
        m["exT"] = np.ascontiguousarray(ex[b].T.astype(f32))
        in_maps.append(m)
    return in_maps


def _run(inputs, **kw):
    nc = _get_nc()
    in_maps = _make_in_maps(inputs)
    res = run_bass_kernel_spmd(nc, in_maps, core_ids=list(range(B)), **kw)
    out = np.stack([res.results[i]["out"] for i in range(B)])
    return out.reshape(B, N, H, W).astype(np.float32), res


def kernel(**inputs):
    out, _ = _run(inputs)
    return out
